# revision 11
# baseline (speedup 1.0000x reference)
"""Causal self-attention kernel for 8 Trainium2 NeuronCores.

Problem: B=4, S=2048, D=1024, H=16, HD=64 (fp32).
  qkv = x @ w_qkv.T ; per-head causal softmax attention ; out @ w_proj.T

Sharding: core c handles batch b = c//2 and head-half hh = c%2 (8 heads).
Each core computes its 8 heads' attention and a partial output projection
(w_proj column slice); the host sums the two partials per batch.

v1 pipeline (vs baseline):
  - all weight DMAs prefetched (2-buf group slices); no mid-kernel PE
    stalls on DMA -> HAM clock gate stays warm (2.4 GHz)
  - Q/K proj with weights stationary (dk-outer, s-windows moving)
  - group g+1 Q/K proj matmuls emission-interleaved into attention(g)'s
    qb loop so the in-order PE fills exp(ACT)-paced gaps
  - epilogue uses reciprocal_approx_fast (DVE custom op, ~5x)
  - V/P/attention-band in bf16 (AV matmuls bf16; PE rate unchanged)
  - out proj w-stationary in bf16 -> yT [e, s]; host transposes
  - PSUM: psA bufs=2 (4 banks) + proj pool (2) + psO (2) = 8
"""

import sys

if "/opt/trn_rl_repo" not in sys.path:
    sys.path.insert(0, "/opt/trn_rl_repo")

import numpy as np

import concourse.tile as tile
from concourse import bacc, mybir

F32 = mybir.dt.float32
F32R = mybir.dt.float32r
BF16 = mybir.dt.bfloat16
EXP = mybir.ActivationFunctionType.Exp

B, S, D = 4, 2048, 1024
H, HD = 16, 64
P = 128
DT = D // P            # 8 d-tiles (contraction tiles for projections)
NHC = 8                # heads per core
NG = NHC // 2          # head pair-groups per core
QB = 4                 # q-blocks of 512
QW = 512               # q-block width
KT = S // P            # 16 k-tiles
XCH = 8                # xT DMA split chunks (along seq)
SCALE = 1.0 / np.sqrt(HD)

_NC = None


def _build(loop_reps=1):
    nc = bacc.Bacc("TRN2", target_bir_lowering=False, debug=False)

    xT = nc.dram_tensor("xT", [D, S], F32R, kind="ExternalInput")
    wqT = nc.dram_tensor("wqT", [D, 512], F32R, kind="ExternalInput")
    wkT = nc.dram_tensor("wkT", [D, 512], F32R, kind="ExternalInput")
    wvT = nc.dram_tensor("wvT", [D, 512], F32R, kind="ExternalInput")
    wpT = nc.dram_tensor("wpT", [512, D], BF16, kind="ExternalInput")
    yT = nc.dram_tensor("yT", [D, S], F32R, kind="ExternalOutput")

    with tile.TileContext(nc) as tc:
        if loop_reps > 1:
            with tc.For_i(0, loop_reps, 1):
                _body(nc, tc, xT, wqT, wkT, wvT, wpT, yT)
        else:
            _body(nc, tc, xT, wqT, wkT, wvT, wpT, yT)
    nc.compile()
    return nc


def _body(nc, tc, xT, wqT, wkT, wvT, wpT, yT):
    with (
        tc.tile_pool(name="big", bufs=1) as big,
        tc.tile_pool(name="wsl", bufs=2) as wsl,
        tc.tile_pool(name="qk", bufs=2) as qkp,
        tc.tile_pool(name="pfull", bufs=2) as pfp,
        tc.tile_pool(name="pband", bufs=1) as pbp,
        tc.tile_pool(name="small", bufs=2) as sp,
        tc.tile_pool(name="ost", bufs=2) as ostp,
        tc.tile_pool(name="psA", bufs=2, space="PSUM") as psA,
        tc.tile_pool(name="psP", bufs=1, space="PSUM") as psP,
        tc.tile_pool(name="psO", bufs=2, space="PSUM") as psO,
    ):
        # ---- persistent loads -------------------------------------------
        # xT split into seq-chunks so compute can start before the full 8MB
        xT_sb = big.tile([P, DT, S], F32R, tag="xT")
        xT_src = xT.ap().rearrange("(o p) s -> p o s", p=P)
        xw = S // XCH
        for c in range(XCH):
            nc.sync.dma_start(
                xT_sb[:, :, c * xw:(c + 1) * xw], xT_src[:, :, c * xw:(c + 1) * xw])

        wvT_sb = big.tile([P, DT, 512], F32R, tag="wv")
        nc.sync.dma_start(wvT_sb, wvT.ap().rearrange("(o p) e -> p o e", p=P))

        # per-group Q/K weight slices, double-buffered + prefetched
        wq_sb = [None] * NG
        wk_sb = [None] * NG

        def load_wqk(g):
            wq_sb[g] = wsl.tile([P, DT, P], F32R, tag="wq", name=f"wq_{g}")
            nc.sync.dma_start(
                wq_sb[g],
                wqT.ap().rearrange("(o p) e -> p o e", p=P)[:, :, g * P:(g + 1) * P],
            )
            wk_sb[g] = wsl.tile([P, DT, P], F32R, tag="wk", name=f"wk_{g}")
            nc.sync.dma_start(
                wk_sb[g],
                wkT.ap().rearrange("(o p) e -> p o e", p=P)[:, :, g * P:(g + 1) * P],
            )

        load_wqk(0)
        load_wqk(1)

        wpT_sb = big.tile([P, 4, D], BF16, tag="wpT")

        # V with a ones column per head: [P, kt, 8 heads * 65] (bf16)
        vaug = big.tile([P, KT, NHC * 65], BF16, tag="vaug")
        ones_cols = vaug.rearrange("p t (h c) -> p t h c", c=65)[:, :, :, 64]
        nc.gpsimd.memset(ones_cols, 1.0)

        # ---- V projection (all 8 heads at once, two s-tiles per psum) ----
        for sp2 in range(KT // 2):
            pv = psA.tile([P, 2, QW], F32, tag="mm", name=f"pv_{sp2}")
            for half in range(2):
                st = 2 * sp2 + half
                for dk in range(DT):
                    nc.tensor.matmul(
                        pv[:, half, :],
                        lhsT=xT_sb[:, dk, st * P:(st + 1) * P],
                        rhs=wvT_sb[:, dk, :],
                        start=(dk == 0), stop=(dk == DT - 1),
                    )
            nc.vector.tensor_copy(
                out=vaug[:, 2 * sp2:2 * sp2 + 2, :]
                    .rearrange("p t (h c) -> p t h c", c=65)[:, :, :, 0:64],
                in_=pv.rearrange("p t (h c) -> p t h c", c=64),
            )

        # wp DMA late (after the x/wv/wq/wk burst; needed only at the end)
        nc.sync.dma_start(wpT_sb, wpT.ap().rearrange("(t p) e -> p t e", p=P))

        # output accumulator O'[do, q] (do = local_head*64 + hd), normalized
        oall = big.tile([P, NG, S], BF16, tag="oall")

        qT = [None] * NG
        kT = [None] * NG

        def emit_proj_sub(g, which, half):
            """Project one 1024-seq half of Q or K for group g.

            Weights stationary (reused across the 2 s-windows); psum tile
            held across the dk contraction loop.
            """
            w_sb = wq_sb[g] if which == "q" else wk_sb[g]
            if half == 0:
                dst = qkp.tile([P, S], F32R, tag=which, name=f"{which}T_{g}")
                if which == "q":
                    qT[g] = dst
                else:
                    kT[g] = dst
            dst = qT[g] if which == "q" else kT[g]
            pt = psP.tile([P, 2, QW], F32, tag="pj", name=f"pj_{which}_{g}_{half}")
            for dk in range(DT):
                for j in range(2):
                    sw = 2 * half + j
                    nc.tensor.matmul(
                        pt[:, j, :],
                        lhsT=w_sb[:, dk, :],
                        rhs=xT_sb[:, dk, sw * QW:(sw + 1) * QW],
                        start=(dk == 0), stop=(dk == DT - 1),
                    )
            nc.vector.tensor_copy(
                out=dst[:, half * 2 * QW:(half + 1) * 2 * QW],
                in_=pt.rearrange("p t q -> p (t q)"),
            )

        # group 0 Q/K projection upfront
        for which in ("q", "k"):
            for half in range(2):
                emit_proj_sub(0, which, half)

        # ---- per head-pair-group: attention (+ next group's proj) -------
        for g in range(NG):
            if g + 2 <= NG - 1:
                load_wqk(g + 2)
            qTg, kTg = qT[g], kT[g]

            for qb in range(QB):
                nkt = 4 * qb + 4  # causal: k-tiles 0 .. 4qb+3
                po = [
                    psO.tile([65, QW], F32, tag="po", name=f"po_{g}_{qb}_{hl}")
                    for hl in range(2)
                ]
                pband = pbp.tile([P, 2, 4, QW], BF16, tag="pband",
                                 name=f"pband_{g}_{qb}")

                for kt in range(nkt):
                    rel = kt - 4 * qb
                    ps2 = psA.tile([P, 2, QW], F32, tag="mm",
                                   name=f"ps_{g}_{qb}_{kt}")
                    for hl in range(2):
                        hp = hl * 64
                        nc.tensor.matmul(
                            ps2[:, hl, :],
                            lhsT=kTg[hp:hp + 64, kt * P:(kt + 1) * P],
                            rhs=qTg[hp:hp + 64, qb * QW:(qb + 1) * QW],
                            start=True, stop=True,
                        )
                    if rel < 0:
                        pp = pfp.tile([P, 2, QW], BF16, tag="pf",
                                      name=f"pf_{g}_{qb}_{kt}")
                        nc.scalar.activation(pp, ps2, EXP, scale=SCALE)
                        for hl in range(2):
                            h = 2 * g + hl
                            nc.tensor.matmul(
                                po[hl],
                                lhsT=vaug[:, kt, h * 65:(h + 1) * 65],
                                rhs=pp[:, hl, :],
                                start=(kt == 0), stop=False,
                            )
                    else:
                        nc.scalar.activation(
                            pband[:, :, rel, :], ps2, EXP, scale=SCALE)

                # zero the causally-invalid region of the diagonal band:
                # keep where  q_col - 128*rel - partition >= 0
                nc.gpsimd.affine_select(
                    out=pband, in_=pband,
                    compare_op=mybir.AluOpType.is_ge, fill=0.0,
                    base=0, channel_multiplier=-1,
                    pattern=[[0, 2], [-P, 4], [1, QW]],
                )

                for rel in range(4):
                    kt = 4 * qb + rel
                    for hl in range(2):
                        h = 2 * g + hl
                        nc.tensor.matmul(
                            po[hl],
                            lhsT=vaug[:, kt, h * 65:(h + 1) * 65],
                            rhs=pband[:, hl, rel, :],
                            start=(kt == 0), stop=(kt == nkt - 1),
                        )

                for hl in range(2):
                    zrow = sp.tile([1, QW], F32, tag="zrow",
                                   name=f"zr_{g}_{qb}_{hl}")
                    nc.vector.tensor_copy(out=zrow, in_=po[hl][64:65, :])
                    recip = sp.tile([1, QW], F32, tag="recip",
                                    name=f"rc_{g}_{qb}_{hl}")
                    nc.vector.reciprocal_approx_fast(recip, zrow)
                    bc = sp.tile([64, QW], F32, tag="bc",
                                 name=f"bc_{g}_{qb}_{hl}")
                    nc.gpsimd.partition_broadcast(bc, recip)
                    nc.vector.tensor_mul(
                        out=oall[hl * 64:(hl + 1) * 64, g, qb * QW:(qb + 1) * QW],
                        in0=po[hl][0:64, :],
                        in1=bc,
                    )

                # interleave next group's Q/K projection into the ACT-paced
                # attention stream (one seq-half per qb iteration)
                if g + 1 <= NG - 1:
                    which, half = (("q", 0), ("q", 1), ("k", 0), ("k", 1))[qb]
                    emit_proj_sub(g + 1, which, half)

        # ---- output projection: yT[e, s] = sum_do wpT[do, e] O'[do, s] --
        for eb in range(D // P):
            for swp in range(2):
                pt = psP.tile([P, 2, QW], F32, tag="pj",
                              name=f"pfin_{eb}_{swp}")
                for t in range(NG):
                    for j in range(2):
                        sw = 2 * swp + j
                        nc.tensor.matmul(
                            pt[:, j, :],
                            lhsT=wpT_sb[:, t, eb * P:(eb + 1) * P],
                            rhs=oall[:, t, sw * QW:(sw + 1) * QW],
                            start=(t == 0), stop=(t == NG - 1),
                        )
                ot = ostp.tile([P, 2 * QW], F32R, tag="ot",
                               name=f"ot_{eb}_{swp}")
                nc.vector.tensor_copy(out=ot, in_=pt.rearrange("p t q -> p (t q)"))
                nc.sync.dma_start(
                    yT.ap()[eb * P:(eb + 1) * P,
                            swp * 2 * QW:(swp + 1) * 2 * QW],
                    ot,
                )


def _get_nc():
    global _NC
    if _NC is None:
        _NC = _build()
    return _NC


def _in_maps(x, w_qkv, w_proj):
    from ml_dtypes import bfloat16

    x = np.asarray(x, dtype=np.float32)
    w_qkv = np.asarray(w_qkv, dtype=np.float32)
    w_proj = np.asarray(w_proj, dtype=np.float32)

    maps = []
    for c in range(8):
        b, hh = c // 2, c % 2
        lo, hi = hh * 512, (hh + 1) * 512
        maps.append({
            "xT": np.ascontiguousarray(x[b].T),
            "wqT": np.ascontiguousarray(w_qkv[lo:hi].T),
            "wkT": np.ascontiguousarray(w_qkv[D + lo:D + hi].T),
            "wvT": np.ascontiguousarray(w_qkv[2 * D + lo:2 * D + hi].T),
            "wpT": np.ascontiguousarray(w_proj[:, lo:hi].T).astype(bfloat16),
        })
    return maps


def kernel(x, w_qkv, w_proj):
    from concourse.bass_utils import run_bass_kernel_spmd

    in_maps = _in_maps(x, w_qkv, w_proj)
    res = run_bass_kernel_spmd(_get_nc(), in_maps, core_ids=list(range(8)))
    out = np.empty((B, S, D), dtype=np.float32)
    for b in range(B):
        out[b] = (res.results[2 * b]["yT"] + res.results[2 * b + 1]["yT"]).T
    return out


# revision 13
# speedup vs baseline: 1.4115x; 1.4115x over previous
"""Causal self-attention kernel for 8 Trainium2 NeuronCores.

Problem: B=4, S=2048, D=1024, H=16, HD=64 (fp32).
  qkv = x @ w_qkv.T ; per-head causal softmax attention ; out @ w_proj.T

Sharding: core c handles batch b = c//2 and head-half hh = c%2 (8 heads).
Each core computes its 8 heads' attention and a partial output projection
(w_proj column slice); the host sums the two partials per batch.

v1 pipeline (vs baseline):
  - all weight DMAs prefetched (2-buf group slices); no mid-kernel PE
    stalls on DMA -> HAM clock gate stays warm (2.4 GHz)
  - Q/K proj with weights stationary (dk-outer, s-windows moving)
  - group g+1 Q/K proj matmuls emission-interleaved into attention(g)'s
    qb loop so the in-order PE fills exp(ACT)-paced gaps
  - epilogue uses reciprocal_approx_fast (DVE custom op, ~5x)
  - V/P/attention-band in bf16 (AV matmuls bf16; PE rate unchanged)
  - out proj w-stationary in bf16 -> yT [e, s]; host transposes
  - PSUM: psA bufs=2 (4 banks) + proj pool (2) + psO (2) = 8
"""

import sys

if "/opt/trn_rl_repo" not in sys.path:
    sys.path.insert(0, "/opt/trn_rl_repo")

import numpy as np

import concourse.tile as tile
from concourse import bacc, mybir

F32 = mybir.dt.float32
F32R = mybir.dt.float32r
BF16 = mybir.dt.bfloat16
EXP = mybir.ActivationFunctionType.Exp

B, S, D = 4, 2048, 1024
H, HD = 16, 64
P = 128
DT = D // P            # 8 d-tiles (contraction tiles for projections)
NHC = 8                # heads per core
NG = NHC // 2          # head pair-groups per core
QB = 4                 # q-blocks of 512
QW = 512               # q-block width
KT = S // P            # 16 k-tiles
XCH = 8                # xT DMA split chunks (along seq)
SCALE = 1.0 / np.sqrt(HD)

_NC = None


def _build(loop_reps=1):
    nc = bacc.Bacc("TRN2", target_bir_lowering=False, debug=False)

    xT = nc.dram_tensor("xT", [D, S], F32R, kind="ExternalInput")
    wqT = nc.dram_tensor("wqT", [D, 512], F32R, kind="ExternalInput")
    wkT = nc.dram_tensor("wkT", [D, 512], F32R, kind="ExternalInput")
    wvT = nc.dram_tensor("wvT", [D, 512], F32R, kind="ExternalInput")
    wpT = nc.dram_tensor("wpT", [512, D], BF16, kind="ExternalInput")
    yT = nc.dram_tensor("yT", [D, S], F32R, kind="ExternalOutput")

    with tile.TileContext(nc) as tc:
        if loop_reps > 1:
            with tc.For_i(0, loop_reps, 1):
                _body(nc, tc, xT, wqT, wkT, wvT, wpT, yT)
        else:
            _body(nc, tc, xT, wqT, wkT, wvT, wpT, yT)
    nc.compile()
    return nc


def _body(nc, tc, xT, wqT, wkT, wvT, wpT, yT):
    with (
        tc.tile_pool(name="big", bufs=1) as big,
        tc.tile_pool(name="wsl", bufs=2) as wsl,
        tc.tile_pool(name="qk", bufs=2) as qkp,
        tc.tile_pool(name="pfull", bufs=2) as pfp,
        tc.tile_pool(name="pband", bufs=1) as pbp,
        tc.tile_pool(name="small", bufs=2) as sp,
        tc.tile_pool(name="small1", bufs=1) as sp1,
        tc.tile_pool(name="ost", bufs=2) as ostp,
        tc.tile_pool(name="psA", bufs=2, space="PSUM") as psA,
        tc.tile_pool(name="psP", bufs=1, space="PSUM") as psP,
        tc.tile_pool(name="psO", bufs=2, space="PSUM") as psO,
    ):
        # ---- persistent loads -------------------------------------------
        # xT split into seq-chunks so compute can start before the full 8MB
        xT_sb = big.tile([P, DT, S], F32R, tag="xT")
        xT_src = xT.ap().rearrange("(o p) s -> p o s", p=P)
        xw = S // XCH
        for c in range(XCH):
            nc.sync.dma_start(
                xT_sb[:, :, c * xw:(c + 1) * xw], xT_src[:, :, c * xw:(c + 1) * xw])

        wvT_sb = big.tile([P, DT, 512], F32R, tag="wv")
        nc.sync.dma_start(wvT_sb, wvT.ap().rearrange("(o p) e -> p o e", p=P))

        # per-group Q/K weight slices, double-buffered + prefetched
        wq_sb = [None] * NG
        wk_sb = [None] * NG

        def load_wqk(g):
            wq_sb[g] = wsl.tile([P, DT, P], F32R, tag="wq", name=f"wq_{g}")
            nc.sync.dma_start(
                wq_sb[g],
                wqT.ap().rearrange("(o p) e -> p o e", p=P)[:, :, g * P:(g + 1) * P],
            )
            wk_sb[g] = wsl.tile([P, DT, P], F32R, tag="wk", name=f"wk_{g}")
            nc.sync.dma_start(
                wk_sb[g],
                wkT.ap().rearrange("(o p) e -> p o e", p=P)[:, :, g * P:(g + 1) * P],
            )

        load_wqk(0)
        load_wqk(1)

        wpT_sb = big.tile([P, 4, D], BF16, tag="wpT")

        # V with a ones column per head: [P, kt, 8 heads * 65]
        vaug = big.tile([P, KT, NHC * 65], F32R, tag="vaug")
        ones_cols = vaug.rearrange("p t (h c) -> p t h c", c=65)[:, :, :, 64]
        nc.gpsimd.memset(ones_cols.bitcast(F32), 1.0)

        # ---- V projection (all 8 heads at once, two s-tiles per psum) ----
        for sp2 in range(KT // 2):
            pv = psA.tile([P, 2, QW], F32, tag="mm", name=f"pv_{sp2}")
            for half in range(2):
                st = 2 * sp2 + half
                for dk in range(DT):
                    nc.tensor.matmul(
                        pv[:, half, :],
                        lhsT=xT_sb[:, dk, st * P:(st + 1) * P],
                        rhs=wvT_sb[:, dk, :],
                        start=(dk == 0), stop=(dk == DT - 1),
                    )
            nc.vector.tensor_copy(
                out=vaug[:, 2 * sp2:2 * sp2 + 2, :]
                    .rearrange("p t (h c) -> p t h c", c=65)[:, :, :, 0:64],
                in_=pv.rearrange("p t (h c) -> p t h c", c=64),
            )

        # wp DMA late (after the x/wv/wq/wk burst; needed only at the end)
        nc.sync.dma_start(wpT_sb, wpT.ap().rearrange("(t p) e -> p t e", p=P))

        # output accumulator O'[do, q] (do = local_head*64 + hd), normalized
        oall = big.tile([P, NG, S], BF16, tag="oall")

        qT = [None] * NG
        kT = [None] * NG

        def emit_proj_sub(g, which, half):
            """Project one 1024-seq half of Q or K for group g.

            Weights stationary (reused across the 2 s-windows); psum tile
            held across the dk contraction loop.
            """
            w_sb = wq_sb[g] if which == "q" else wk_sb[g]
            if half == 0:
                dst = qkp.tile([P, S], BF16, tag=which, name=f"{which}T_{g}")
                if which == "q":
                    qT[g] = dst
                else:
                    kT[g] = dst
            dst = qT[g] if which == "q" else kT[g]
            pt = psP.tile([P, 2, QW], F32, tag="pj", name=f"pj_{which}_{g}_{half}")
            for dk in range(DT):
                for j in range(2):
                    sw = 2 * half + j
                    nc.tensor.matmul(
                        pt[:, j, :],
                        lhsT=w_sb[:, dk, :],
                        rhs=xT_sb[:, dk, sw * QW:(sw + 1) * QW],
                        start=(dk == 0), stop=(dk == DT - 1),
                    )
            nc.vector.tensor_copy(
                out=dst[:, half * 2 * QW:(half + 1) * 2 * QW],
                in_=pt.rearrange("p t q -> p (t q)"),
            )

        # group 0 Q/K projection upfront
        for which in ("q", "k"):
            for half in range(2):
                emit_proj_sub(0, which, half)

        # ---- per head-pair-group: attention (+ next group's proj) -------
        for g in range(NG):
            if g + 2 <= NG - 1:
                load_wqk(g + 2)
            qTg, kTg = qT[g], kT[g]

            for qb in range(QB):
                nkt = 4 * qb + 4  # causal: k-tiles 0 .. 4qb+3
                po = [
                    psO.tile([65, QW], F32, tag="po", name=f"po_{g}_{qb}_{hl}")
                    for hl in range(2)
                ]
                pband = pbp.tile([P, 2, 4, QW], F32R, tag="pband",
                                 name=f"pband_{g}_{qb}")

                for kt in range(nkt):
                    rel = kt - 4 * qb
                    ps2 = psA.tile([P, 2, QW], F32, tag="mm",
                                   name=f"ps_{g}_{qb}_{kt}")
                    v0 = 0 if rel < 0 else P * rel  # causal col restriction
                    for hl in range(2):
                        hp = hl * 64
                        nc.tensor.matmul(
                            ps2[:, hl, v0:],
                            lhsT=kTg[hp:hp + 64, kt * P:(kt + 1) * P],
                            rhs=qTg[hp:hp + 64, qb * QW + v0:(qb + 1) * QW],
                            start=True, stop=True,
                        )
                    if rel < 0:
                        pp = pfp.tile([P, 2, QW], F32R, tag="pf",
                                      name=f"pf_{g}_{qb}_{kt}")
                        nc.scalar.activation(pp, ps2, EXP, scale=SCALE)
                        for hl in range(2):
                            h = 2 * g + hl
                            nc.tensor.matmul(
                                po[hl],
                                lhsT=vaug[:, kt, h * 65:(h + 1) * 65],
                                rhs=pp[:, hl, :],
                                start=(kt == 0), stop=False,
                            )
                    else:
                        # band: exp only the causally-reachable columns, then
                        # zero the invalid triangle edge per-rel (pipelines
                        # with the next rel's exp instead of one big select)
                        nc.scalar.activation(
                            pband[:, :, rel, v0:], ps2[:, :, v0:], EXP,
                            scale=SCALE)
                        w0 = min(v0, QW - 2 * P)  # AV reads from here
                        w1 = min(v0 + P, QW)
                        nc.gpsimd.affine_select(
                            out=pband[:, :, rel, w0:QW if rel == 3 else w1],
                            in_=pband[:, :, rel, w0:QW if rel == 3 else w1],
                            compare_op=mybir.AluOpType.is_ge, fill=0.0,
                            base=w0 - P * rel, channel_multiplier=-1,
                            pattern=[[0, 2], [1, (QW if rel == 3 else w1) - w0]],
                        )

                for rel in range(4):
                    kt = 4 * qb + rel
                    av0 = min(P * rel, QW - 2 * P)
                    for hl in range(2):
                        h = 2 * g + hl
                        nc.tensor.matmul(
                            po[hl][:, av0:],
                            lhsT=vaug[:, kt, h * 65:(h + 1) * 65],
                            rhs=pband[:, hl, rel, av0:],
                            start=(kt == 0), stop=(kt == nkt - 1),
                        )

                for hl in range(2):
                    zrow = sp1.tile([1, QW], F32, tag="zrow",
                                   name=f"zr_{g}_{qb}_{hl}")
                    nc.vector.tensor_copy(out=zrow, in_=po[hl][64:65, :])
                    recip = sp1.tile([1, QW], F32, tag="recip",
                                    name=f"rc_{g}_{qb}_{hl}")
                    nc.vector.reciprocal_approx_fast(recip, zrow)
                    bc = sp.tile([64, QW], F32, tag="bc",
                                 name=f"bc_{g}_{qb}_{hl}")
                    nc.gpsimd.partition_broadcast(bc, recip)
                    nc.vector.tensor_mul(
                        out=oall[hl * 64:(hl + 1) * 64, g, qb * QW:(qb + 1) * QW],
                        in0=po[hl][0:64, :],
                        in1=bc,
                    )

                # interleave next group's Q/K projection into the ACT-paced
                # attention stream (one seq-half per qb iteration)
                if g + 1 <= NG - 1:
                    which, half = (("q", 0), ("q", 1), ("k", 0), ("k", 1))[qb]
                    emit_proj_sub(g + 1, which, half)

        # ---- output projection: yT[e, s] = sum_do wpT[do, e] O'[do, s] --
        for eb in range(D // P):
            for swp in range(2):
                pt = psP.tile([P, 2, QW], F32, tag="pj",
                              name=f"pfin_{eb}_{swp}")
                for t in range(NG):
                    for j in range(2):
                        sw = 2 * swp + j
                        nc.tensor.matmul(
                            pt[:, j, :],
                            lhsT=wpT_sb[:, t, eb * P:(eb + 1) * P],
                            rhs=oall[:, t, sw * QW:(sw + 1) * QW],
                            start=(t == 0), stop=(t == NG - 1),
                        )
                for j in range(2):
                    ot = ostp.tile([P, QW], F32R, tag="ot",
                                   name=f"ot_{eb}_{swp}_{j}")
                    nc.vector.tensor_copy(out=ot, in_=pt[:, j, :])
                    sw = 2 * swp + j
                    nc.sync.dma_start(
                        yT.ap()[eb * P:(eb + 1) * P, sw * QW:(sw + 1) * QW],
                        ot,
                    )


def _get_nc():
    global _NC
    if _NC is None:
        _NC = _build()
    return _NC


def _in_maps(x, w_qkv, w_proj):
    from ml_dtypes import bfloat16

    x = np.asarray(x, dtype=np.float32)
    w_qkv = np.asarray(w_qkv, dtype=np.float32)
    w_proj = np.asarray(w_proj, dtype=np.float32)

    maps = []
    for c in range(8):
        b, hh = c // 2, c % 2
        lo, hi = hh * 512, (hh + 1) * 512
        maps.append({
            "xT": np.ascontiguousarray(x[b].T),
            "wqT": np.ascontiguousarray(w_qkv[lo:hi].T),
            "wkT": np.ascontiguousarray(w_qkv[D + lo:D + hi].T),
            "wvT": np.ascontiguousarray(w_qkv[2 * D + lo:2 * D + hi].T),
            "wpT": np.ascontiguousarray(w_proj[:, lo:hi].T).astype(bfloat16),
        })
    return maps


def kernel(x, w_qkv, w_proj):
    from concourse.bass_utils import run_bass_kernel_spmd

    in_maps = _in_maps(x, w_qkv, w_proj)
    res = run_bass_kernel_spmd(_get_nc(), in_maps, core_ids=list(range(8)))
    out = np.empty((B, S, D), dtype=np.float32)
    for b in range(B):
        out[b] = (res.results[2 * b]["yT"] + res.results[2 * b + 1]["yT"]).T
    return out


# revision 14
# speedup vs baseline: 1.6756x; 1.1870x over previous
"""Causal self-attention kernel for 8 Trainium2 NeuronCores.

Problem: B=4, S=2048, D=1024, H=16, HD=64 (fp32).
  qkv = x @ w_qkv.T ; per-head causal softmax attention ; out @ w_proj.T

Sharding: core c handles batch b = c//2 and head-half hh = c%2 (8 heads).
Each core computes its 8 heads' attention and a partial output projection
(w_proj column slice); the host sums the two partials per batch.

v1 pipeline (vs baseline):
  - all weight DMAs prefetched (2-buf group slices); no mid-kernel PE
    stalls on DMA -> HAM clock gate stays warm (2.4 GHz)
  - Q/K proj with weights stationary (dk-outer, s-windows moving)
  - group g+1 Q/K proj matmuls emission-interleaved into attention(g)'s
    qb loop so the in-order PE fills exp(ACT)-paced gaps
  - epilogue uses reciprocal_approx_fast (DVE custom op, ~5x)
  - V/P/attention-band in bf16 (AV matmuls bf16; PE rate unchanged)
  - out proj w-stationary in bf16 -> yT [e, s]; host transposes
  - PSUM: psA bufs=2 (4 banks) + proj pool (2) + psO (2) = 8
"""

import sys

if "/opt/trn_rl_repo" not in sys.path:
    sys.path.insert(0, "/opt/trn_rl_repo")

import numpy as np

import concourse.tile as tile
from concourse import bacc, mybir

F32 = mybir.dt.float32
F32R = mybir.dt.float32r
BF16 = mybir.dt.bfloat16
EXP = mybir.ActivationFunctionType.Exp

B, S, D = 4, 2048, 1024
H, HD = 16, 64
P = 128
DT = D // P            # 8 d-tiles (contraction tiles for projections)
NHC = 8                # heads per core
NG = NHC // 2          # head pair-groups per core
QB = 4                 # q-blocks of 512
QW = 512               # q-block width
KT = S // P            # 16 k-tiles
XCH = 8                # xT DMA split chunks (along seq)
SCALE = 1.0 / np.sqrt(HD)

_NC = None


def _build(loop_reps=1):
    nc = bacc.Bacc("TRN2", target_bir_lowering=False, debug=False)

    xT = nc.dram_tensor("xT", [D, S], BF16, kind="ExternalInput")
    wqT = nc.dram_tensor("wqT", [D, 512], BF16, kind="ExternalInput")
    wkT = nc.dram_tensor("wkT", [D, 512], BF16, kind="ExternalInput")
    wvT = nc.dram_tensor("wvT", [D, 512], BF16, kind="ExternalInput")
    wpT = nc.dram_tensor("wpT", [512, D], BF16, kind="ExternalInput")
    yT = nc.dram_tensor("yT", [D, S], F32R, kind="ExternalOutput")

    with tile.TileContext(nc) as tc:
        if loop_reps > 1:
            with tc.For_i(0, loop_reps, 1):
                _body(nc, tc, xT, wqT, wkT, wvT, wpT, yT)
        else:
            _body(nc, tc, xT, wqT, wkT, wvT, wpT, yT)
    nc.compile()
    return nc


def _body(nc, tc, xT, wqT, wkT, wvT, wpT, yT):
    with (
        tc.tile_pool(name="big", bufs=1) as big,
        tc.tile_pool(name="wsl", bufs=2) as wsl,
        tc.tile_pool(name="qk", bufs=2) as qkp,
        tc.tile_pool(name="pfull", bufs=2) as pfp,
        tc.tile_pool(name="pband", bufs=1) as pbp,
        tc.tile_pool(name="small", bufs=2) as sp,
        tc.tile_pool(name="small1", bufs=1) as sp1,
        tc.tile_pool(name="ost", bufs=2) as ostp,
        tc.tile_pool(name="psA", bufs=2, space="PSUM") as psA,
        tc.tile_pool(name="psP", bufs=1, space="PSUM") as psP,
        tc.tile_pool(name="psO", bufs=2, space="PSUM") as psO,
    ):
        # ---- persistent loads -------------------------------------------
        # xT split into seq-chunks so compute can start before the full load;
        # wv + first chunks lead so the V projection starts ~8us in
        xT_sb = big.tile([P, DT, S], BF16, tag="xT")
        xT_src = xT.ap().rearrange("(o p) s -> p o s", p=P)
        xw = S // XCH

        def load_xchunk(c):
            nc.sync.dma_start(
                xT_sb[:, :, c * xw:(c + 1) * xw], xT_src[:, :, c * xw:(c + 1) * xw])

        wvT_sb = big.tile([P, DT, 512], BF16, tag="wv")
        nc.sync.dma_start(wvT_sb, wvT.ap().rearrange("(o p) e -> p o e", p=P))
        load_xchunk(0)
        load_xchunk(1)

        # per-group Q/K weight slices, double-buffered + prefetched
        wq_sb = [None] * NG
        wk_sb = [None] * NG

        def load_wqk(g):
            wq_sb[g] = wsl.tile([P, DT, P], BF16, tag="wq", name=f"wq_{g}")
            nc.sync.dma_start(
                wq_sb[g],
                wqT.ap().rearrange("(o p) e -> p o e", p=P)[:, :, g * P:(g + 1) * P],
            )
            wk_sb[g] = wsl.tile([P, DT, P], BF16, tag="wk", name=f"wk_{g}")
            nc.sync.dma_start(
                wk_sb[g],
                wkT.ap().rearrange("(o p) e -> p o e", p=P)[:, :, g * P:(g + 1) * P],
            )

        load_wqk(0)
        for c in range(2, XCH):
            load_xchunk(c)
        load_wqk(1)

        wpT_sb = big.tile([P, 4, D], BF16, tag="wpT")

        # V with a ones column per head: [P, kt, 8 heads * 65]
        vaug = big.tile([P, KT, NHC * 65], F32R, tag="vaug")
        ones_cols = vaug.rearrange("p t (h c) -> p t h c", c=65)[:, :, :, 64]
        nc.gpsimd.memset(ones_cols.bitcast(F32), 1.0)

        # ---- V projection (all 8 heads at once, two s-tiles per psum) ----
        for sp2 in range(KT // 2):
            pv = psA.tile([P, 2, QW], F32, tag="mm", name=f"pv_{sp2}")
            for half in range(2):
                st = 2 * sp2 + half
                for dk in range(DT):
                    nc.tensor.matmul(
                        pv[:, half, :],
                        lhsT=xT_sb[:, dk, st * P:(st + 1) * P],
                        rhs=wvT_sb[:, dk, :],
                        start=(dk == 0), stop=(dk == DT - 1),
                    )
            nc.vector.tensor_copy(
                out=vaug[:, 2 * sp2:2 * sp2 + 2, :]
                    .rearrange("p t (h c) -> p t h c", c=65)[:, :, :, 0:64],
                in_=pv.rearrange("p t (h c) -> p t h c", c=64),
            )

        # wp DMA late (after the x/wv/wq/wk burst; needed only at the end)
        nc.sync.dma_start(wpT_sb, wpT.ap().rearrange("(t p) e -> p t e", p=P))

        # output accumulator O'[do, q] (do = local_head*64 + hd), normalized
        oall = big.tile([P, NG, S], BF16, tag="oall")

        qT = [None] * NG
        kT = [None] * NG

        def emit_proj_sub(g, which, half):
            """Project one 1024-seq half of Q or K for group g.

            Weights stationary (reused across the 2 s-windows); psum tile
            held across the dk contraction loop.
            """
            w_sb = wq_sb[g] if which == "q" else wk_sb[g]
            if half == 0:
                dst = qkp.tile([P, S], BF16, tag=which, name=f"{which}T_{g}")
                if which == "q":
                    qT[g] = dst
                else:
                    kT[g] = dst
            dst = qT[g] if which == "q" else kT[g]
            pt = psP.tile([P, 2, QW], F32, tag="pj", name=f"pj_{which}_{g}_{half}")
            for dk in range(DT):
                for j in range(2):
                    sw = 2 * half + j
                    nc.tensor.matmul(
                        pt[:, j, :],
                        lhsT=w_sb[:, dk, :],
                        rhs=xT_sb[:, dk, sw * QW:(sw + 1) * QW],
                        start=(dk == 0), stop=(dk == DT - 1),
                    )
            nc.vector.tensor_copy(
                out=dst[:, half * 2 * QW:(half + 1) * 2 * QW],
                in_=pt.rearrange("p t q -> p (t q)"),
            )

        def emit_outproj_sw(sw):
            # yT[e, sw-window] = sum_t wpT[:, t, e].T @ oall[:, t, sw-window]
            # (emitted per s-window so g3's attention slack absorbs it)
            for ebp in range(NG):
                pt = psP.tile([P, 2, QW], F32, tag="pj",
                              name=f"pfin_{sw}_{ebp}")
                for t in range(NG):
                    for j in range(2):
                        eb = 2 * ebp + j
                        nc.tensor.matmul(
                            pt[:, j, :],
                            lhsT=wpT_sb[:, t, eb * P:(eb + 1) * P],
                            rhs=oall[:, t, sw * QW:(sw + 1) * QW],
                            start=(t == 0), stop=(t == NG - 1),
                        )
                for j in range(2):
                    eb = 2 * ebp + j
                    ot = ostp.tile([P, QW], F32R, tag="ot",
                                   name=f"ot_{sw}_{eb}")
                    nc.vector.tensor_copy(out=ot, in_=pt[:, j, :])
                    nc.sync.dma_start(
                        yT.ap()[eb * P:(eb + 1) * P, sw * QW:(sw + 1) * QW],
                        ot,
                    )

        # group 0 Q/K projection upfront
        for which in ("q", "k"):
            for half in range(2):
                emit_proj_sub(0, which, half)

        # ---- per head-pair-group: attention (+ next group's proj) -------
        for g in range(NG):
            if g + 2 <= NG - 1:
                load_wqk(g + 2)
            qTg, kTg = qT[g], kT[g]

            for qb in range(QB):
                nkt = 4 * qb + 4  # causal: k-tiles 0 .. 4qb+3
                po = [
                    psO.tile([65, QW], F32, tag="po", name=f"po_{g}_{qb}_{hl}")
                    for hl in range(2)
                ]
                pband = pbp.tile([P, 2, 4, QW], F32R, tag="pband",
                                 name=f"pband_{g}_{qb}")

                for kt in range(nkt):
                    rel = kt - 4 * qb
                    ps2 = psA.tile([P, 2, QW], F32, tag="mm",
                                   name=f"ps_{g}_{qb}_{kt}")
                    v0 = 0 if rel < 0 else P * rel  # causal col restriction
                    for hl in range(2):
                        hp = hl * 64
                        nc.tensor.matmul(
                            ps2[:, hl, v0:],
                            lhsT=kTg[hp:hp + 64, kt * P:(kt + 1) * P],
                            rhs=qTg[hp:hp + 64, qb * QW + v0:(qb + 1) * QW],
                            start=True, stop=True,
                        )
                    if rel < 0:
                        pp = pfp.tile([P, 2, QW], F32R, tag="pf",
                                      name=f"pf_{g}_{qb}_{kt}")
                        nc.scalar.activation(pp, ps2, EXP, scale=SCALE)
                        for hl in range(2):
                            h = 2 * g + hl
                            nc.tensor.matmul(
                                po[hl],
                                lhsT=vaug[:, kt, h * 65:(h + 1) * 65],
                                rhs=pp[:, hl, :],
                                start=(kt == 0), stop=False,
                            )
                    else:
                        # band: exp only the causally-reachable columns, then
                        # zero the invalid triangle edge per-rel (pipelines
                        # with the next rel's exp instead of one big select)
                        nc.scalar.activation(
                            pband[:, :, rel, v0:], ps2[:, :, v0:], EXP,
                            scale=SCALE)
                        w0 = min(v0, QW - 2 * P)  # AV reads from here
                        w1 = min(v0 + P, QW)
                        nc.gpsimd.affine_select(
                            out=pband[:, :, rel, w0:QW if rel == 3 else w1],
                            in_=pband[:, :, rel, w0:QW if rel == 3 else w1],
                            compare_op=mybir.AluOpType.is_ge, fill=0.0,
                            base=w0 - P * rel, channel_multiplier=-1,
                            pattern=[[0, 2], [1, (QW if rel == 3 else w1) - w0]],
                        )

                for rel in range(4):
                    kt = 4 * qb + rel
                    av0 = min(P * rel, QW - 2 * P)
                    for hl in range(2):
                        h = 2 * g + hl
                        nc.tensor.matmul(
                            po[hl][:, av0:],
                            lhsT=vaug[:, kt, h * 65:(h + 1) * 65],
                            rhs=pband[:, hl, rel, av0:],
                            start=(kt == 0), stop=(kt == nkt - 1),
                        )

                for hl in range(2):
                    zrow = sp1.tile([1, QW], F32, tag="zrow",
                                   name=f"zr_{g}_{qb}_{hl}")
                    nc.vector.tensor_copy(out=zrow, in_=po[hl][64:65, :])
                    recip = sp1.tile([1, QW], F32, tag="recip",
                                    name=f"rc_{g}_{qb}_{hl}")
                    nc.vector.reciprocal_approx_fast(recip, zrow)
                    bc = sp.tile([64, QW], F32, tag="bc",
                                 name=f"bc_{g}_{qb}_{hl}")
                    nc.gpsimd.partition_broadcast(bc, recip)
                    nc.vector.tensor_mul(
                        out=oall[hl * 64:(hl + 1) * 64, g, qb * QW:(qb + 1) * QW],
                        in0=po[hl][0:64, :],
                        in1=bc,
                    )

                # interleave next group's Q/K projection (g<3) or the
                # output projection for this qb's s-window (g=3) into the
                # ACT-paced attention stream
                if g + 1 <= NG - 1:
                    which, half = (("q", 0), ("q", 1), ("k", 0), ("k", 1))[qb]
                    emit_proj_sub(g + 1, which, half)
                else:
                    emit_outproj_sw(qb)



def _get_nc():
    global _NC
    if _NC is None:
        _NC = _build()
    return _NC


def _in_maps(x, w_qkv, w_proj):
    from ml_dtypes import bfloat16

    x = np.asarray(x, dtype=np.float32)
    w_qkv = np.asarray(w_qkv, dtype=np.float32)
    w_proj = np.asarray(w_proj, dtype=np.float32)

    maps = []
    for c in range(8):
        b, hh = c // 2, c % 2
        lo, hi = hh * 512, (hh + 1) * 512
        maps.append({
            "xT": np.ascontiguousarray(x[b].T).astype(bfloat16),
            "wqT": np.ascontiguousarray(w_qkv[lo:hi].T).astype(bfloat16),
            "wkT": np.ascontiguousarray(w_qkv[D + lo:D + hi].T).astype(bfloat16),
            "wvT": np.ascontiguousarray(w_qkv[2 * D + lo:2 * D + hi].T).astype(bfloat16),
            "wpT": np.ascontiguousarray(w_proj[:, lo:hi].T).astype(bfloat16),
        })
    return maps


def kernel(x, w_qkv, w_proj):
    from concourse.bass_utils import run_bass_kernel_spmd

    in_maps = _in_maps(x, w_qkv, w_proj)
    res = run_bass_kernel_spmd(_get_nc(), in_maps, core_ids=list(range(8)))
    out = np.empty((B, S, D), dtype=np.float32)
    for b in range(B):
        out[b] = (res.results[2 * b]["yT"] + res.results[2 * b + 1]["yT"]).T
    return out
